# revision 30
# baseline (speedup 1.0000x reference)
"""Trainium2 Bass kernel for a two-window sparse causal self-attention block.

Model (B=2, T=2048, C=1024):
  - 8 "short" heads: d_qk=32,  window 256
  - 8 "long"  heads: d_qk=128, window 1024
  - value/output head dim 64, output projection C x C.

Sharding (8 cores): data-parallel over batch (2) x head-parallel over head
groups (4). Core c = 4*b + g handles batch b and heads {2g, 2g+1} of both the
short and long sets. Each core computes its 4 heads' attention plus the
corresponding 256 rows of Wproj, producing a partial [T, C] output; the host
sums the 4 partials per batch element.

Device-side design notes:
  - float32r matmuls everywhere: full PE rate (1 cycle/row at N>=256) vs 2
    cycles/row for fp32, ~1.5e-4 matmul relative error.
  - everything is computed in "transposed" orientation so no on-device
    transposes are needed: host passes xT [C, T]; projections give qT/kT
    [d, T] and v [T, dv]; scores sT[k, q] = kT.T @ qT; yT[dv, q] = v_aug.T @
    pT with a ones column in v so row 64 of yT accumulates softmax sums.
  - queries processed in groups of 512 (4 blocks) so score/AV matmuls run at
    N=512; the causal band mask is ADDITIVE (0 / -1e30) and is accumulated
    into the scores PSUM tile by an identity-weight matmul (start=False), so
    no vector/gpsimd mask multiplies are needed; exp of masked entries
    underflows to exact 0.
  - exp skips the max-subtraction: inputs are well-scaled (|scores| < ~10).
  - normalization: per-head reciprocal of the sums row via the fast DVE
    approx; per head-pair one rank-2 indicator matmul broadcasts the two
    reciprocal rows across the 128 output partitions, then a DVE multiply
    writes normalized outputs straight from PSUM.
  - software pipelining: AV matmuls for score-pair j are emitted after the
    scores of pair j+1; the output projection of query group g is emitted
    after the attention of group g+1; normalization matmuls are emitted
    inside the following head's score stream. This keeps the PE queue free
    of cross-engine waits.
"""

import math

import numpy as np

import concourse.bass as bass
import concourse.mybir as mybir
import concourse.tile as tile
from concourse.bass_utils import run_bass_kernel_spmd

F32 = mybir.dt.float32
F32R = mybir.dt.float32r

B, T, C = 2, 2048, 1024
HS, DS = 8, 32
HL, DL = 8, 128
HD = 64
WIN_S, WIN_L = 256, 1024
NT = T // 128    # 16 t-blocks
NCB = C // 128   # 8 c-blocks
NG = T // 512    # 4 query groups
VW = HD + 1      # v columns + ones column for softmax sums
N_CORES = 8
NEG = -1.0e30


def _split_waits(nc: bass.Bass) -> int:
    """Walrus in this env accepts at most 1 sync wait per instruction.
    Hoist extra waits onto same-engine InstNoOp instructions placed just
    before the owning instruction (same-engine program order preserves the
    blocking semantics)."""
    import bass_rust

    n_added = 0
    for f in nc.m.functions:
        for bb in f.blocks:
            insts = bb.instructions
            if not any(inst.sync_info and len(inst.sync_info.on_wait) > 1
                       for inst in insts):
                continue
            new = []
            for inst in insts:
                si = inst.sync_info
                waits = list(si.on_wait) if si else []
                if len(waits) > 1:
                    for i, w in enumerate(waits[:-1]):
                        nop = mybir.InstNoOp(
                            name=f"{inst.name}_hw{i}",
                            sync_info=bass_rust.SyncInfo(on_wait=[w], on_update=[]),
                            bass_nofuse=True,
                            engine=inst.engine,
                        )
                        new.append(nop)
                        n_added += 1
                    inst.sync_info = bass_rust.SyncInfo(
                        on_wait=waits[-1:], on_update=list(si.on_update))
                new.append(inst)
            bb.instructions = new
    return n_added


def _patch_tile_drain():
    """This walrus build rejects >1 sync wait on the TileContext tail drain
    ("Too many sync wait commands"). Re-emit the drain's waits as individual
    wait_ge instructions on the sync engine."""
    import bass_rust
    from concourse.tile import ScopedClock, TileContext

    def _drain_and_barrier(self, tick_clock, wait_clock):
        nc = self.nc
        drain_inst = nc.sync.drain()
        wait_clock.add_sem_waits(
            drain_inst.ins, ScopedClock({None: tick_clock.global_clock})
        )
        si = drain_inst.ins.sync_info
        waits = list(si.on_wait) if si is not None else []
        if len(waits) > 1:
            drain_inst.ins.sync_info = bass_rust.SyncInfo(on_wait=[], on_update=[])
            sems = {h.name: h for h in self.sems.allocated().values()}
            for w in waits:
                nc.sync.wait_ge(sems[w.ant_name], w.wait_value)
        nc.all_engine_barrier()
        popped = nc._tile_sem_poison_stack.pop()
        assert popped is self._sem_poison
        nc.clear_and_free_semaphores(list(self.sems.allocated().values()))
        nc.all_engine_barrier()

    TileContext._drain_and_barrier = _drain_and_barrier


_patch_tile_drain()


def _build_program() -> bass.Bass:
    nc = bass.Bass()

    xt_d = nc.dram_tensor("xt", [C, T], F32, kind="ExternalInput")
    wsqk_d = nc.dram_tensor("wsqk", [C, 128], F32, kind="ExternalInput")
    wql_d = nc.dram_tensor("wql", [C, 256], F32, kind="ExternalInput")
    wkl_d = nc.dram_tensor("wkl", [C, 256], F32, kind="ExternalInput")
    wv_d = nc.dram_tensor("wv", [C, 256], F32, kind="ExternalInput")
    wp_d = nc.dram_tensor("wp", [256, C], F32, kind="ExternalInput")
    bs_d = nc.dram_tensor("band_s", [128, WIN_S + 896], F32, kind="ExternalInput")
    bl_d = nc.dram_tensor("band_l", [128, WIN_L + 896], F32, kind="ExternalInput")
    id_d = nc.dram_tensor("ident", [128, 128], F32, kind="ExternalInput")
    i2_d = nc.dram_tensor("ind2", [65, 64], F32, kind="ExternalInput")
    out_d = nc.dram_tensor("out", [T, C], F32, kind="ExternalOutput")

    scale_s = 1.0 / math.sqrt(DS)
    scale_l = 1.0 / math.sqrt(DL)

    with tile.TileContext(nc) as tc:
        with (
            tc.tile_pool(name="const", bufs=1) as const,
            tc.tile_pool(name="qkp", bufs=1) as qkp,
            tc.tile_pool(name="vp", bufs=1) as vp,
            tc.tile_pool(name="xtp", bufs=2) as xtp,
            tc.tile_pool(name="ptp", bufs=6) as ptp,
            tc.tile_pool(name="ytp", bufs=2) as ytp,
            tc.tile_pool(name="obp", bufs=3) as obp,
            tc.tile_pool(name="smallp", bufs=2) as smallp,
            tc.tile_pool(name="bigps", bufs=2, space="PSUM") as bigps,
            tc.tile_pool(name="yhps", bufs=2, space="PSUM") as yhps,
            tc.tile_pool(name="rbps", bufs=2, space="PSUM") as rbps,
        ):
            # ---- projection outputs (persist across both stages) ----
            qts = qkp.tile([64, T], F32R, tag="qts", name="qts")
            kts = qkp.tile([64, T], F32R, tag="kts", name="kts")
            qtl = [qkp.tile([128, T], F32R, tag=f"qtl{h}", name=f"qtl{h}") for h in range(2)]
            ktl = [qkp.tile([128, T], F32R, tag=f"ktl{h}", name=f"ktl{h}") for h in range(2)]
            vt = [vp.tile([128, NT * VW], F32R, tag=f"vt{i}", name=f"vt{i}") for i in range(4)]
            # ones column of each v block (for softmax sums), via Pool memset
            for i in range(4):
                v3 = vt[i][:, :].bitcast(F32).rearrange("p (nt vw) -> p nt vw", vw=VW)
                nc.gpsimd.memset(v3[:, :, HD], 1.0)

            # ---- DMAs ordered so first-needed data lands first ----
            wsqk = const.tile([128, NCB, 128], F32R, tag="wsqk", name="wsqk")
            nc.sync.dma_start(wsqk[:], wsqk_d[:, :].bitcast(F32R).rearrange("(cb p) d -> p cb d", p=128))

            def load_x(tch):
                xs = []
                for cb in range(NCB):
                    t = xtp.tile([128, 512], F32R, tag=f"xt{cb}", name=f"xt{cb}")
                    nc.sync.dma_start(
                        t[:],
                        xt_d[cb * 128:(cb + 1) * 128,
                             tch * 512:(tch + 1) * 512].bitcast(F32R))
                    xs.append(t)
                return xs

            xs0 = load_x(0)

            wql = const.tile([128, NCB, 256], F32R, tag="wql", name="wql")
            nc.sync.dma_start(wql[:], wql_d[:, :].bitcast(F32R).rearrange("(cb p) d -> p cb d", p=128))
            wkl = const.tile([128, NCB, 256], F32R, tag="wkl", name="wkl")
            nc.sync.dma_start(wkl[:], wkl_d[:, :].bitcast(F32R).rearrange("(cb p) d -> p cb d", p=128))
            wv = const.tile([128, NCB, 256], F32R, tag="wv", name="wv")
            nc.sync.dma_start(wv[:], wv_d[:, :].bitcast(F32R).rearrange("(cb p) d -> p cb d", p=128))

            xs1 = load_x(1)

            band_s = const.tile([128, WIN_S + 896], F32R, tag="band_s", name="band_s")
            nc.sync.dma_start(band_s[:], bs_d[:, :].bitcast(F32R))
            band_l = const.tile([128, WIN_L + 896], F32R, tag="band_l", name="band_l")
            nc.sync.dma_start(band_l[:], bl_d[:, :].bitcast(F32R))
            ident = const.tile([128, 128], F32R, tag="ident", name="ident")
            nc.sync.dma_start(ident[:], id_d[:, :].bitcast(F32R))
            ind2 = const.tile([65, 64], F32R, tag="ind2", name="ind2")
            nc.sync.dma_start(ind2[:], i2_d[:, :].bitcast(F32R))

            # ================= stage A: projections =================
            proj_jobs = [(wsqk, None, None)]
            for h in range(2):
                proj_jobs.append((wql, h, qtl[h]))
                proj_jobs.append((wkl, h, ktl[h]))

            def stage_a(tch, xs):
                for ji, (w, h, dst) in enumerate(proj_jobs):
                    ps = bigps.tile([128, 1024], F32, tag="bigps", name="bigps")
                    for cb in range(NCB):
                        lhsT = w[:, cb, :] if h is None else w[:, cb, h * 128:(h + 1) * 128]
                        nc.tensor.matmul(
                            ps[:, 0:512], lhsT, xs[cb][:, :],
                            start=(cb == 0), stop=(cb == NCB - 1),
                        )
                    sl = (slice(None), slice(tch * 512, (tch + 1) * 512))
                    if dst is None:
                        nc.scalar.copy(qts[sl], ps[0:64, 0:512])
                        nc.scalar.copy(kts[sl], ps[64:128, 0:512])
                    elif ji % 2 == 0:
                        nc.vector.tensor_copy(dst[sl], ps[:, 0:512])
                    else:
                        nc.scalar.copy(dst[sl], ps[:, 0:512])
                for tbl in range(4):
                    tb = 4 * tch + tbl
                    ps = bigps.tile([128, 1024], F32, tag="bigps", name="bigps")
                    for cb in range(NCB):
                        nc.tensor.matmul(
                            ps[:, 0:256], xs[cb][:, tbl * 128:(tbl + 1) * 128], wv[:, cb, :],
                            start=(cb == 0), stop=(cb == NCB - 1),
                        )
                    for i in range(4):
                        eng = nc.vector.tensor_copy if i % 2 == 0 else nc.scalar.copy
                        eng(vt[i][:, tb * VW: tb * VW + HD], ps[:, i * 64:(i + 1) * 64])

            stage_a(0, xs0)
            xs2 = load_x(2)
            stage_a(1, xs1)
            xs3 = load_x(3)
            stage_a(2, xs2)
            stage_a(3, xs3)

            # stage-B projection weights (first needed mid stage B)
            wp0 = const.tile([128, C], F32R, tag="wp0", name="wp0")
            nc.sync.dma_start(wp0[:], wp_d[0:128, :].bitcast(F32R))
            wp1 = const.tile([128, C], F32R, tag="wp1", name="wp1")
            nc.sync.dma_start(wp1[:], wp_d[128:256, :].bitcast(F32R))

            # ============ stage B: attention + output projection ============
            pending = []

            def flush_pending():
                while pending:
                    pending.pop(0)()

            def head_stream(qg, kt_ap, qt_ap, v_tile, win, scale, band, r_out,
                            out_yvs, do_flush):
                """Generator: emits one score-pair (+exp, lagged AV) per
                next(); appends the head's yv tile to out_yvs when done."""
                q0 = qg * 512
                kb_lo = max(0, q0 - win) // 128
                kb_hi = (q0 + 384) // 128
                kbs = list(range(kb_lo, kb_hi + 1))
                pairs = [kbs[j:j + 2] for j in range(0, len(kbs), 2)]
                flush_at = min(1, len(pairs) - 1)
                yh = yhps.tile([VW, 512], F32, tag="yh", name="yh")
                av_q = []
                n_av = 0
                n_tot = len(kbs)

                def emit_av():
                    nonlocal n_av
                    kb2, pt2, psl2 = av_q.pop(0)
                    nc.tensor.matmul(yh[:], v_tile[:, kb2 * VW:(kb2 + 1) * VW],
                                     pt2[psl2],
                                     start=(n_av == 0),
                                     stop=(n_av == n_tot - 1))
                    n_av += 1

                for pj, pair in enumerate(pairs):
                    wdt = 512 * len(pair)
                    st = bigps.tile([128, 1024], F32, tag="bigps", name="bigps")
                    for jj, kb in enumerate(pair):
                        delta = kb * 128 - q0
                        needs_mask = not (512 - win <= delta <= -128)
                        psl = (slice(None), slice(jj * 512, (jj + 1) * 512))
                        nc.tensor.matmul(st[psl], kt_ap(kb), qt_ap,
                                         start=True, stop=not needs_mask)
                        if needs_mask:
                            off = 384 - delta
                            nc.tensor.matmul(st[psl], ident[:, :],
                                             band[:, off: off + 512],
                                             start=False, stop=True)
                    pt = ptp.tile([128, 1024], F32R, tag="pt", name="pt")
                    nc.scalar.activation(
                        pt[:, 0:wdt], st[:, 0:wdt],
                        mybir.ActivationFunctionType.Exp, scale=scale)
                    if do_flush and pj == flush_at:
                        flush_pending()
                    for jj, kb in enumerate(pair):
                        av_q.append((kb, pt, (slice(None), slice(jj * 512, (jj + 1) * 512))))
                    # AV lags two score pairs behind so exp latency is hidden
                    while len(av_q) > 4:
                        emit_av()
                    yield
                while av_q:
                    emit_av()
                yv = smallp.tile([64, 512], F32, tag="yv", name="yv", bufs=4)
                nc.vector.tensor_copy(yv[:, :], yh[0:HD, :])
                nc.vector.tensor_copy(r_out, yh[HD:HD + 1, :])
                out_yvs.append(yv)

            def drive_pair(ga, gb):
                """Alternate two head generators pair-by-pair."""
                live = [ga, gb]
                while live:
                    for g in list(live):
                        try:
                            next(g)
                        except StopIteration:
                            live.remove(g)

            def mk_finalize(s2, yvs, yts_tile):
                def fin():
                    nc.scalar.activation(s2[:, :], s2[:, :],
                                         mybir.ActivationFunctionType.Ln)
                    r2 = smallp.tile([33, 512], F32R, tag="r2", name="r2")
                    nc.scalar.activation(r2[:, :], s2[:, :],
                                         mybir.ActivationFunctionType.Exp,
                                         scale=-1.0)
                    with nc.allow_low_precision(reason="f32r rounding of attn out"):
                        for hh in range(2):
                            rbp = rbps.tile([64, 512], F32, tag="rbp", name="rbp")
                            nc.tensor.matmul(rbp[:, :], ind2[32 * hh:32 * hh + 1, :],
                                             r2[32 * hh:32 * hh + 1, :],
                                             start=True, stop=True)
                            nc.vector.tensor_mul(
                                yts_tile[64 * hh:64 * hh + 64, :],
                                yvs[hh][:, :], rbp[:, :])
                return fin

            def emit_proj(qg, yts):
                q0 = qg * 512
                for sub in range(4):
                    qs = q0 + sub * 128
                    ssl = (slice(None), slice(sub * 128, (sub + 1) * 128))
                    ob = obp.tile([128, 1024], F32, tag="ob", name="ob")
                    for nh in range(2):
                        po = bigps.tile([128, 1024], F32, tag="bigps", name="bigps")
                        nc.tensor.matmul(po[:, 0:512], yts[0][ssl],
                                         wp0[:, nh * 512:(nh + 1) * 512],
                                         start=True, stop=False)
                        nc.tensor.matmul(po[:, 0:512], yts[1][ssl],
                                         wp1[:, nh * 512:(nh + 1) * 512],
                                         start=False, stop=True)
                        nc.vector.tensor_copy(ob[:, nh * 512:(nh + 1) * 512], po[:, 0:512])
                    nc.sync.dma_start(out_d[qs: qs + 128, :], ob[:])

            prev_yts = None
            for qg in range(NG):
                q0 = qg * 512
                yts = [ytp.tile([128, 512], F32R, tag=f"yts{i}", name=f"yts{i}")
                       for i in range(2)]
                # last qg runs the long pair first so the tail chain is short
                pair_order = (0, 1) if qg < NG - 1 else (1, 0)
                for pi in pair_order:
                    s2 = smallp.tile([33, 512], F32, tag="s2", name="s2")
                    yvs = []
                    gens = []
                    for hh in range(2):
                        s_ap = s2[32 * hh:32 * hh + 1, :]
                        if pi == 0:
                            h = hh
                            g = head_stream(
                                qg,
                                lambda kb, h=h: kts[32 * h: 32 * h + 32, kb * 128:(kb + 1) * 128],
                                qts[32 * h: 32 * h + 32, q0: q0 + 512],
                                vt[h], WIN_S, scale_s, band_s, s_ap,
                                yvs, hh == 0)
                        else:
                            h = hh
                            g = head_stream(
                                qg,
                                lambda kb, h=h: ktl[h][:, kb * 128:(kb + 1) * 128],
                                qtl[h][:, q0: q0 + 512],
                                vt[2 + h], WIN_L, scale_l, band_l, s_ap,
                                yvs, hh == 0)
                        gens.append(g)
                    drive_pair(gens[0], gens[1])
                    pending.append(mk_finalize(s2, yvs, yts[pi]))
                if prev_yts is not None:
                    emit_proj(qg - 1, prev_yts)
                prev_yts = yts
            flush_pending()
            emit_proj(NG - 1, prev_yts)

    return nc


_PROGRAM = None


def _get_program() -> bass.Bass:
    global _PROGRAM
    if _PROGRAM is None:
        _PROGRAM = _build_program()
        _split_waits(_PROGRAM)
    return _PROGRAM


def _band_image(win: int) -> np.ndarray:
    """[128, win+896] additive mask: 0 where (u - 384 - r) in [0, win),
    else -1e30."""
    u = np.arange(win + 896)[None, :]
    r = np.arange(128)[:, None]
    d = u - 384 - r
    bad = (d < 0) | (d >= win)
    return np.where(bad, np.float32(NEG), np.float32(0.0)).astype(np.float32)


def make_in_maps(x, Wqk_short, Wv_short, Wqk_long, Wv_long, Wproj):
    """Host-side sharding: per-core input dict for core c = 4*b + g."""
    x = np.ascontiguousarray(np.asarray(x, dtype=np.float32))
    Wqk_short = np.asarray(Wqk_short, dtype=np.float32)
    Wv_short = np.asarray(Wv_short, dtype=np.float32)
    Wqk_long = np.asarray(Wqk_long, dtype=np.float32)
    Wv_long = np.asarray(Wv_long, dtype=np.float32)
    Wproj = np.asarray(Wproj, dtype=np.float32)
    assert x.shape == (B, T, C)

    xts = [np.ascontiguousarray(x[b].T) for b in range(B)]
    band_s = _band_image(WIN_S)
    band_l = _band_image(WIN_L)
    ident = np.eye(128, dtype=np.float32)
    ind2 = np.ones((65, 64), dtype=np.float32)
    in_maps = []
    for c in range(N_CORES):
        b, g = divmod(c, 4)
        wsqk = np.ascontiguousarray(np.concatenate(
            [Wqk_short[:, g * 64:(g + 1) * 64],
             Wqk_short[:, 256 + g * 64: 256 + (g + 1) * 64]], axis=1))
        wql = np.ascontiguousarray(Wqk_long[:, g * 256:(g + 1) * 256])
        wkl = np.ascontiguousarray(Wqk_long[:, 1024 + g * 256: 1024 + (g + 1) * 256])
        wv = np.ascontiguousarray(np.concatenate(
            [Wv_short[:, g * 128:(g + 1) * 128],
             Wv_long[:, g * 128:(g + 1) * 128]], axis=1))
        wp = np.ascontiguousarray(np.concatenate(
            [Wproj[g * 128:(g + 1) * 128, :],
             Wproj[512 + g * 128: 512 + (g + 1) * 128, :]], axis=0))
        in_maps.append({
            "xt": xts[b], "wsqk": wsqk, "wql": wql, "wkl": wkl, "wv": wv, "wp": wp,
            "band_s": band_s, "band_l": band_l, "ident": ident, "ind2": ind2,
        })
    return in_maps


def gather(results) -> np.ndarray:
    out = np.empty((B, T, C), dtype=np.float32)
    for b in range(B):
        acc = np.zeros((T, C), dtype=np.float64)
        for g in range(4):
            acc += results[4 * b + g]["out"]
        out[b] = acc.astype(np.float32)
    return out


def kernel(x, Wqk_short, Wv_short, Wqk_long, Wv_long, Wproj, **run_kwargs):
    nc = _get_program()
    in_maps = make_in_maps(x, Wqk_short, Wv_short, Wqk_long, Wv_long, Wproj)
    res = run_bass_kernel_spmd(nc, in_maps, core_ids=list(range(N_CORES)), **run_kwargs)
    out = gather(res.results)
    if run_kwargs:
        kernel.last_results = res
    return out


# revision 38
# speedup vs baseline: 1.1545x; 1.1545x over previous
"""Trainium2 Bass kernel for a two-window sparse causal self-attention block.

Model (B=2, T=2048, C=1024):
  - 8 "short" heads: d_qk=32,  window 256
  - 8 "long"  heads: d_qk=128, window 1024
  - value/output head dim 64, output projection C x C.

Sharding (8 cores): data-parallel over batch (2) x head-parallel over head
groups (4). Core c = 4*b + g handles batch b and heads {2g, 2g+1} of both the
short and long sets. Each core computes its 4 heads' attention plus the
corresponding 256 rows of Wproj, producing a partial [T, C] output; the host
sums the 4 partials per batch element.

Device-side design notes:
  - float32r matmuls everywhere: full PE rate (1 cycle/row at N>=256) vs 2
    cycles/row for fp32, ~1.5e-4 matmul relative error.
  - everything is computed in "transposed" orientation so no on-device
    transposes are needed: host passes xT [C, T]; projections give qT/kT
    [d, T] and v [T, dv]; scores sT[k, q] = kT.T @ qT; yT[dv, q] = v_aug.T @
    pT with a ones column in v so row 64 of yT accumulates softmax sums.
  - queries processed in groups of 512 (4 blocks) so score/AV matmuls run at
    N=512; the causal band mask is ADDITIVE (0 / -1e30) and is accumulated
    into the scores PSUM tile by an identity-weight matmul (start=False), so
    no vector/gpsimd mask multiplies are needed; exp of masked entries
    underflows to exact 0.
  - exp skips the max-subtraction: inputs are well-scaled (|scores| < ~10).
  - normalization: per-head reciprocal of the sums row via the fast DVE
    approx; per head-pair one rank-2 indicator matmul broadcasts the two
    reciprocal rows across the 128 output partitions, then a DVE multiply
    writes normalized outputs straight from PSUM.
  - software pipelining: AV matmuls for score-pair j are emitted after the
    scores of pair j+1; the output projection of query group g is emitted
    after the attention of group g+1; normalization matmuls are emitted
    inside the following head's score stream. This keeps the PE queue free
    of cross-engine waits.
"""

import math

import numpy as np

import concourse.bass as bass
import concourse.mybir as mybir
import concourse.tile as tile
from concourse.bass_utils import run_bass_kernel_spmd

F32 = mybir.dt.float32
F32R = mybir.dt.float32r

B, T, C = 2, 2048, 1024
HS, DS = 8, 32
HL, DL = 8, 128
HD = 64
WIN_S, WIN_L = 256, 1024
NT = T // 128    # 16 t-blocks
NCB = C // 128   # 8 c-blocks
NG = T // 512    # 4 query groups
VW = HD + 1      # v columns + ones column for softmax sums
N_CORES = 8
NEG = -1.0e30


def _split_waits(nc: bass.Bass) -> int:
    """Walrus in this env accepts at most 1 sync wait per instruction.
    Hoist extra waits onto same-engine InstNoOp instructions placed just
    before the owning instruction (same-engine program order preserves the
    blocking semantics)."""
    import bass_rust

    n_added = 0
    for f in nc.m.functions:
        for bb in f.blocks:
            insts = bb.instructions
            if not any(inst.sync_info and len(inst.sync_info.on_wait) > 1
                       for inst in insts):
                continue
            new = []
            for inst in insts:
                si = inst.sync_info
                waits = list(si.on_wait) if si else []
                if len(waits) > 1:
                    for i, w in enumerate(waits[:-1]):
                        nop = mybir.InstNoOp(
                            name=f"{inst.name}_hw{i}",
                            sync_info=bass_rust.SyncInfo(on_wait=[w], on_update=[]),
                            bass_nofuse=True,
                            engine=inst.engine,
                        )
                        new.append(nop)
                        n_added += 1
                    inst.sync_info = bass_rust.SyncInfo(
                        on_wait=waits[-1:], on_update=list(si.on_update))
                new.append(inst)
            bb.instructions = new
    return n_added


def _patch_tile_drain():
    """This walrus build rejects >1 sync wait on the TileContext tail drain
    ("Too many sync wait commands"). Re-emit the drain's waits as individual
    wait_ge instructions on the sync engine."""
    import bass_rust
    from concourse.tile import ScopedClock, TileContext

    def _drain_and_barrier(self, tick_clock, wait_clock):
        nc = self.nc
        drain_inst = nc.sync.drain()
        wait_clock.add_sem_waits(
            drain_inst.ins, ScopedClock({None: tick_clock.global_clock})
        )
        si = drain_inst.ins.sync_info
        waits = list(si.on_wait) if si is not None else []
        if len(waits) > 1:
            drain_inst.ins.sync_info = bass_rust.SyncInfo(on_wait=[], on_update=[])
            sems = {h.name: h for h in self.sems.allocated().values()}
            for w in waits:
                nc.sync.wait_ge(sems[w.ant_name], w.wait_value)
        nc.all_engine_barrier()
        popped = nc._tile_sem_poison_stack.pop()
        assert popped is self._sem_poison
        nc.clear_and_free_semaphores(list(self.sems.allocated().values()))
        nc.all_engine_barrier()

    TileContext._drain_and_barrier = _drain_and_barrier


_patch_tile_drain()


def _build_program() -> bass.Bass:
    nc = bass.Bass()

    xt_d = nc.dram_tensor("xt", [C, T], F32, kind="ExternalInput")
    wsqk_d = nc.dram_tensor("wsqk", [C, 128], F32, kind="ExternalInput")
    wql_d = nc.dram_tensor("wql", [C, 256], F32, kind="ExternalInput")
    wkl_d = nc.dram_tensor("wkl", [C, 256], F32, kind="ExternalInput")
    wv_d = nc.dram_tensor("wv", [C, 256], F32, kind="ExternalInput")
    wp_d = nc.dram_tensor("wp", [256, C], F32, kind="ExternalInput")
    bs_d = nc.dram_tensor("band_s", [128, WIN_S + 896], F32, kind="ExternalInput")
    bl_d = nc.dram_tensor("band_l", [128, WIN_L + 896], F32, kind="ExternalInput")
    id_d = nc.dram_tensor("ident", [128, 128], F32, kind="ExternalInput")
    i2_d = nc.dram_tensor("ind2", [65, 64], F32, kind="ExternalInput")
    out_d = nc.dram_tensor("out", [T, C], F32, kind="ExternalOutput")

    scale_s = 1.0 / math.sqrt(DS)
    scale_l = 1.0 / math.sqrt(DL)

    with tile.TileContext(nc) as tc:
        with (
            tc.tile_pool(name="const", bufs=1) as const,
            tc.tile_pool(name="qkp", bufs=1) as qkp,
            tc.tile_pool(name="vp", bufs=1) as vp,
            tc.tile_pool(name="bigps", bufs=2, space="PSUM") as bigps,
            tc.tile_pool(name="yhps", bufs=2, space="PSUM") as yhps,
            tc.tile_pool(name="rbps", bufs=2, space="PSUM") as rbps,
        ):
            # warm the PE clock up during the initial DMA wait: matmuls on a
            # locally-memset tile start as soon as the framework preamble ends
            wdum = const.tile([128, 512], F32R, tag="wdum", name="wdum")
            nc.gpsimd.memset(wdum[:, :].bitcast(F32), 1.0)
            for wi in range(16):
                wps = bigps.tile([128, 1024], F32, tag="bigps", name="bigps")
                nc.tensor.matmul(wps[:, 0:512], wdum[:, 0:128], wdum[:, :],
                                 start=True, stop=True)

            # ---- projection outputs (persist across both stages) ----
            # short-head q/k are zero-padded to K=128 partitions: K=32
            # matmuls make the PE DVFS governor drop the clock for whole
            # regions; padded contraction keeps it at full speed for free.
            qtsp = [qkp.tile([128, T], F32R, tag=f"qtsp{h}", name=f"qtsp{h}") for h in range(2)]
            ktsp = [qkp.tile([128, T], F32R, tag=f"ktsp{h}", name=f"ktsp{h}") for h in range(2)]
            for tl in qtsp + ktsp:
                for p0 in (32, 64, 96):
                    nc.gpsimd.memset(tl[p0:p0 + 32, :].bitcast(F32), 0.0)
            qtl = [qkp.tile([128, T], F32R, tag=f"qtl{h}", name=f"qtl{h}") for h in range(2)]
            ktl = [qkp.tile([128, T], F32R, tag=f"ktl{h}", name=f"ktl{h}") for h in range(2)]
            vt = [vp.tile([128, NT * VW], F32R, tag=f"vt{i}", name=f"vt{i}") for i in range(4)]
            # ones column of each v block (for softmax sums), via Pool memset
            for i in range(4):
                v3 = vt[i][:, :].bitcast(F32).rearrange("p (nt vw) -> p nt vw", vw=VW)
                nc.gpsimd.memset(v3[:, :, HD], 1.0)

            # ---- DMAs ordered so first-needed data lands first ----
            wsqk = const.tile([128, NCB, 128], F32R, tag="wsqk", name="wsqk")
            nc.sync.dma_start(wsqk[:], wsqk_d[:, :].bitcast(F32R).rearrange("(cb p) d -> p cb d", p=128))

            # ================= stage A: projections =================
            with (
                tc.tile_pool(name="awp", bufs=1) as awp,
                tc.tile_pool(name="xtp", bufs=2) as xtp,
            ):
                def load_x(tch):
                    t = xtp.tile([128, NCB, 512], F32R, tag="xt", name="xt")
                    for cb in range(NCB):
                        nc.sync.dma_start(
                            t[:, cb, :],
                            xt_d[cb * 128:(cb + 1) * 128,
                                 tch * 512:(tch + 1) * 512].bitcast(F32R))
                    return t

                xs0 = load_x(0)

                wql = awp.tile([128, NCB, 256], F32R, tag="wql", name="wql")
                nc.sync.dma_start(wql[:], wql_d[:, :].bitcast(F32R).rearrange("(cb p) d -> p cb d", p=128))
                wkl = awp.tile([128, NCB, 256], F32R, tag="wkl", name="wkl")
                nc.sync.dma_start(wkl[:], wkl_d[:, :].bitcast(F32R).rearrange("(cb p) d -> p cb d", p=128))
                wv = awp.tile([128, NCB, 256], F32R, tag="wv", name="wv")
                nc.sync.dma_start(wv[:], wv_d[:, :].bitcast(F32R).rearrange("(cb p) d -> p cb d", p=128))

                xs1 = load_x(1)

                proj_jobs = [(None, None, None)]
                for h in range(2):
                    proj_jobs.append((wql, h, qtl[h]))
                    proj_jobs.append((wkl, h, ktl[h]))

                def stage_a(tch, xs):
                    for ji, (w, h, dst) in enumerate(proj_jobs):
                        ps = bigps.tile([128, 1024], F32, tag="bigps", name="bigps")
                        for cb in range(NCB):
                            lhsT = (wsqk[:, cb, :] if dst is None
                                    else w[:, cb, h * 128:(h + 1) * 128])
                            nc.tensor.matmul(
                                ps[:, 0:512], lhsT, xs[:, cb, :],
                                start=(cb == 0), stop=(cb == NCB - 1),
                            )
                        sl = (slice(None), slice(tch * 512, (tch + 1) * 512))
                        psl = (slice(0, 32), sl[1])
                        if dst is None:
                            nc.scalar.copy(qtsp[0][psl], ps[0:32, 0:512])
                            nc.vector.tensor_copy(qtsp[1][psl], ps[32:64, 0:512])
                            nc.scalar.copy(ktsp[0][psl], ps[64:96, 0:512])
                            nc.vector.tensor_copy(ktsp[1][psl], ps[96:128, 0:512])
                        else:
                            sl0 = (slice(None), slice(tch * 512, tch * 512 + 256))
                            sl1 = (slice(None), slice(tch * 512 + 256, (tch + 1) * 512))
                            nc.vector.tensor_copy(dst[sl0], ps[:, 0:256])
                            nc.scalar.copy(dst[sl1], ps[:, 256:512])
                    for tbl in range(4):
                        tb = 4 * tch + tbl
                        ps = bigps.tile([128, 1024], F32, tag="bigps", name="bigps")
                        for cb in range(NCB):
                            nc.tensor.matmul(
                                ps[:, 0:256], xs[:, cb, tbl * 128:(tbl + 1) * 128], wv[:, cb, :],
                                start=(cb == 0), stop=(cb == NCB - 1),
                            )
                        for i in range(4):
                            eng = nc.vector.tensor_copy if i % 2 == 0 else nc.scalar.copy
                            eng(vt[i][:, tb * VW: tb * VW + HD], ps[:, i * 64:(i + 1) * 64])

                stage_a(0, xs0)
                xs2 = load_x(2)

                band_s = const.tile([128, WIN_S + 896], F32R, tag="band_s", name="band_s")
                nc.sync.dma_start(band_s[:], bs_d[:, :].bitcast(F32R))
                band_l = const.tile([128, WIN_L + 896], F32R, tag="band_l", name="band_l")
                nc.sync.dma_start(band_l[:], bl_d[:, :].bitcast(F32R))
                ident = const.tile([128, 128], F32R, tag="ident", name="ident")
                nc.sync.dma_start(ident[:], id_d[:, :].bitcast(F32R))
                ind2 = const.tile([65, 64], F32R, tag="ind2", name="ind2")
                nc.sync.dma_start(ind2[:], i2_d[:, :].bitcast(F32R))

                stage_a(1, xs1)
                xs3 = load_x(3)
                stage_a(2, xs2)
                stage_a(3, xs3)

                # stage-B projection weights (first needed mid stage B)
                wp0 = const.tile([128, C], F32R, tag="wp0", name="wp0")
                nc.sync.dma_start(wp0[:], wp_d[0:128, :].bitcast(F32R))
                wp1 = const.tile([128, C], F32R, tag="wp1", name="wp1")
                nc.sync.dma_start(wp1[:], wp_d[128:256, :].bitcast(F32R))

            # ============ stage B: attention + output projection ============
            pending = []

            def flush_pending():
                while pending:
                    pending.pop(0)()

            def head_stream(qg, kt_ap, qt_ap, v_tile, win, scale, band, r_out,
                            out_yvs, do_flush):
                """Generator: emits one score-pair (+exp, lagged AV) per
                next(); appends the head's yv tile to out_yvs when done."""
                q0 = qg * 512
                kb_lo = max(0, q0 - win) // 128
                kb_hi = (q0 + 384) // 128
                kbs = list(range(kb_lo, kb_hi + 1))
                pairs = [kbs[j:j + 2] for j in range(0, len(kbs), 2)]
                flush_at = min(2, len(pairs) - 1)
                yh = yhps.tile([VW, 512], F32, tag="yh", name="yh")
                av_q = []
                n_av = 0
                n_tot = len(kbs)

                def emit_av():
                    nonlocal n_av
                    kb2, pt2, psl2 = av_q.pop(0)
                    nc.tensor.matmul(yh[:], v_tile[:, kb2 * VW:(kb2 + 1) * VW],
                                     pt2[psl2],
                                     start=(n_av == 0),
                                     stop=(n_av == n_tot - 1))
                    n_av += 1

                for pj, pair in enumerate(pairs):
                    wdt = 512 * len(pair)
                    st = bigps.tile([128, 1024], F32, tag="bigps", name="bigps")
                    for jj, kb in enumerate(pair):
                        delta = kb * 128 - q0
                        needs_mask = not (512 - win <= delta <= -128)
                        psl = (slice(None), slice(jj * 512, (jj + 1) * 512))
                        nc.tensor.matmul(st[psl], kt_ap(kb), qt_ap,
                                         start=True, stop=not needs_mask)
                        if needs_mask:
                            off = 384 - delta
                            nc.tensor.matmul(st[psl], ident[:, :],
                                             band[:, off: off + 512],
                                             start=False, stop=True)
                    pt = ptp.tile([128, 1024], F32R, tag="pt", name="pt")
                    nc.scalar.activation(
                        pt[:, 0:wdt], st[:, 0:wdt],
                        mybir.ActivationFunctionType.Exp, scale=scale)
                    if do_flush and pj == flush_at:
                        flush_pending()
                    for jj, kb in enumerate(pair):
                        av_q.append((kb, pt, (slice(None), slice(jj * 512, (jj + 1) * 512))))
                    # AV lags two score pairs behind so exp latency is hidden
                    while len(av_q) > 4:
                        emit_av()
                    yield
                while av_q:
                    emit_av()
                yv = smallp.tile([64, 512], F32, tag="yv", name="yv", bufs=4)
                nc.vector.tensor_copy(yv[:, :], yh[0:HD, :])
                nc.vector.tensor_copy(r_out, yh[HD:HD + 1, :])
                out_yvs.append(yv)

            def drive_pair(ga, gb):
                """Alternate two head generators pair-by-pair."""
                live = [ga, gb]
                while live:
                    for g in list(live):
                        try:
                            next(g)
                        except StopIteration:
                            live.remove(g)

            def mk_finalize(s2, yvs, yts_tile):
                def fin():
                    nc.scalar.activation(s2[:, :], s2[:, :],
                                         mybir.ActivationFunctionType.Ln)
                    r2 = smallp.tile([33, 512], F32R, tag="r2", name="r2")
                    nc.scalar.activation(r2[:, :], s2[:, :],
                                         mybir.ActivationFunctionType.Exp,
                                         scale=-1.0)
                    with nc.allow_low_precision(reason="f32r rounding of attn out"):
                        for hh in range(2):
                            rbp = rbps.tile([64, 512], F32, tag="rbp", name="rbp")
                            nc.tensor.matmul(rbp[:, :], ind2[32 * hh:32 * hh + 1, :],
                                             r2[32 * hh:32 * hh + 1, :],
                                             start=True, stop=True)
                            nc.vector.tensor_mul(
                                yts_tile[64 * hh:64 * hh + 64, :],
                                yvs[hh][:, :], rbp[:, :])
                return fin

            def emit_proj(qg, yts):
                q0 = qg * 512
                for sg in range(2):
                    ob = obp.tile([128, 2 * 1024], F32, tag="ob", name="ob")
                    for si in range(2):
                        sub = 2 * sg + si
                        ssl = (slice(None), slice(sub * 128, (sub + 1) * 128))
                        for nh in range(2):
                            po = bigps.tile([128, 1024], F32, tag="bigps", name="bigps")
                            nc.tensor.matmul(po[:, 0:512], yts[0][ssl],
                                             wp0[:, nh * 512:(nh + 1) * 512],
                                             start=True, stop=False)
                            nc.tensor.matmul(po[:, 0:512], yts[1][ssl],
                                             wp1[:, nh * 512:(nh + 1) * 512],
                                             start=False, stop=True)
                            osl = (slice(None),
                                   slice(si * 1024 + nh * 512, si * 1024 + (nh + 1) * 512))
                            if nh == 0:
                                nc.vector.tensor_copy(ob[osl], po[:, 0:512])
                            else:
                                nc.scalar.copy(ob[osl], po[:, 0:512])
                    qs = q0 + sg * 256
                    nc.sync.dma_start(
                        out_d[qs: qs + 256, :].rearrange("(s p) c -> p s c", p=128),
                        ob[:, :].rearrange("p (s c) -> p s c", c=1024))
                    ob = None

            prev_yts = None
            for qg in range(NG):
                q0 = qg * 512
                yts = [ytp.tile([128, 512], F32R, tag=f"yts{i}", name=f"yts{i}")
                       for i in range(2)]
                # last qg runs the long pair first so the tail chain is short
                pair_order = (0, 1) if qg < NG - 1 else (1, 0)
                for pi in pair_order:
                    s2 = smallp.tile([33, 512], F32, tag="s2", name="s2")
                    yvs = []
                    gens = []
                    for hh in range(2):
                        s_ap = s2[32 * hh:32 * hh + 1, :]
                        if pi == 0:
                            h = hh
                            g = head_stream(
                                qg,
                                lambda kb, h=h: kts[32 * h: 32 * h + 32, kb * 128:(kb + 1) * 128],
                                qts[32 * h: 32 * h + 32, q0: q0 + 512],
                                vt[h], WIN_S, scale_s, band_s, s_ap,
                                yvs, hh == 0)
                        else:
                            h = hh
                            g = head_stream(
                                qg,
                                lambda kb, h=h: ktl[h][:, kb * 128:(kb + 1) * 128],
                                qtl[h][:, q0: q0 + 512],
                                vt[2 + h], WIN_L, scale_l, band_l, s_ap,
                                yvs, hh == 0)
                        gens.append(g)
                    drive_pair(gens[0], gens[1])
                    pending.append(mk_finalize(s2, yvs, yts[pi]))
                if prev_yts is not None:
                    emit_proj(qg - 1, prev_yts)
                prev_yts = yts
            flush_pending()
            emit_proj(NG - 1, prev_yts)

    return nc


_PROGRAM = None


def _get_program() -> bass.Bass:
    global _PROGRAM
    if _PROGRAM is None:
        _PROGRAM = _build_program()
        _split_waits(_PROGRAM)
    return _PROGRAM


def _band_image(win: int) -> np.ndarray:
    """[128, win+896] additive mask: 0 where (u - 384 - r) in [0, win),
    else -1e30."""
    u = np.arange(win + 896)[None, :]
    r = np.arange(128)[:, None]
    d = u - 384 - r
    bad = (d < 0) | (d >= win)
    return np.where(bad, np.float32(NEG), np.float32(0.0)).astype(np.float32)


def make_in_maps(x, Wqk_short, Wv_short, Wqk_long, Wv_long, Wproj):
    """Host-side sharding: per-core input dict for core c = 4*b + g."""
    x = np.ascontiguousarray(np.asarray(x, dtype=np.float32))
    Wqk_short = np.asarray(Wqk_short, dtype=np.float32)
    Wv_short = np.asarray(Wv_short, dtype=np.float32)
    Wqk_long = np.asarray(Wqk_long, dtype=np.float32)
    Wv_long = np.asarray(Wv_long, dtype=np.float32)
    Wproj = np.asarray(Wproj, dtype=np.float32)
    assert x.shape == (B, T, C)

    xts = [np.ascontiguousarray(x[b].T) for b in range(B)]
    band_s = _band_image(WIN_S)
    band_l = _band_image(WIN_L)
    ident = np.eye(128, dtype=np.float32)
    ind2 = np.ones((65, 64), dtype=np.float32)
    in_maps = []
    for c in range(N_CORES):
        b, g = divmod(c, 4)
        wsqk = np.ascontiguousarray(np.concatenate(
            [Wqk_short[:, g * 64:(g + 1) * 64],
             Wqk_short[:, 256 + g * 64: 256 + (g + 1) * 64]], axis=1))
        wql = np.ascontiguousarray(Wqk_long[:, g * 256:(g + 1) * 256])
        wkl = np.ascontiguousarray(Wqk_long[:, 1024 + g * 256: 1024 + (g + 1) * 256])
        wv = np.ascontiguousarray(np.concatenate(
            [Wv_short[:, g * 128:(g + 1) * 128],
             Wv_long[:, g * 128:(g + 1) * 128]], axis=1))
        wp = np.ascontiguousarray(np.concatenate(
            [Wproj[g * 128:(g + 1) * 128, :],
             Wproj[512 + g * 128: 512 + (g + 1) * 128, :]], axis=0))
        in_maps.append({
            "xt": xts[b], "wsqk": wsqk, "wql": wql, "wkl": wkl, "wv": wv, "wp": wp,
            "band_s": band_s, "band_l": band_l, "ident": ident, "ind2": ind2,
        })
    return in_maps


def gather(results) -> np.ndarray:
    out = np.empty((B, T, C), dtype=np.float32)
    for b in range(B):
        acc = np.zeros((T, C), dtype=np.float64)
        for g in range(4):
            acc += results[4 * b + g]["out"]
        out[b] = acc.astype(np.float32)
    return out


def kernel(x, Wqk_short, Wv_short, Wqk_long, Wv_long, Wproj, **run_kwargs):
    nc = _get_program()
    in_maps = make_in_maps(x, Wqk_short, Wv_short, Wqk_long, Wv_long, Wproj)
    res = run_bass_kernel_spmd(nc, in_maps, core_ids=list(range(N_CORES)), **run_kwargs)
    out = gather(res.results)
    if run_kwargs:
        kernel.last_results = res
    return out
